# revision 42
# baseline (speedup 1.0000x reference)
"""Trainium2 Bass kernel for nn_PositionalEncoding_61151744360729.

out[b, s, n, :] = x[b, s, n, :] + ||x[b, s+1, n, :] - x[b, s, n, :]||_2
(with distance 0 at s = S-1).

Sharding: data-parallel on batch across 8 NeuronCores (64 batches/core).

Device layout: fp16 end-to-end, c-planar. Host repacks x to, per
(batch, seq-half) partition, [3 coord planes][SH+1 frames][26 nodes]
(nodes padded 25->26 so the one-frame shift is 52B = 4B-aligned and all
DVE tensor_tensor ops hit the 2x perf mode; fp16 I/O halves HBM traffic
vs fp32).

Engine split per chunk: DVE does the 3 per-plane frame-shift subtracts
and the 3 per-plane broadcast-adds; ACT squares each plane (packing
26->25) and takes the final sqrt; the 3-plane sum runs on the
otherwise-idle PE as identity matmuls accumulating into PSUM (the sqrt
doubles as the PSUM->SBUF drain); outputs DMA out per plane. A
one-chunk software pipeline keeps the in-order ACT queue from stalling
on in-flight matmuls, and chunk sizes taper (32 frames at the ends,
64 mid) to shrink pipeline fill and the final DMA drain.
"""

import sys
from contextlib import ExitStack

for _p in ("/opt/trn_rl_repo", "/root/.axon_site/_ro/trn_rl_repo"):
    if _p not in sys.path:
        sys.path.insert(0, _p)

import numpy as np

import concourse.bass as bass
import concourse.tile as tile
from concourse import bacc, mybir
from concourse.bass_utils import run_bass_kernel_spmd

B, S, N, C = 512, 1024, 25, 3
NCORES = 8
BC = B // NCORES           # 64 batches per core
H = 2                      # sequence halves -> 128 partitions
SH = S // H                # 512 frames per half
P = H * BC                 # 128 partitions
NP = 26                    # nodes padded to 26 (4B-aligned frame stride)
IN_PLANE = (SH + 1) * NP   # input elems per coord plane per partition
OUT_PLANE = SH * NP        # output elems per plane per partition
IN_FLAT = P * C * IN_PLANE
OUT_FLAT = P * C * OUT_PLANE
PSUM_W = 416               # matmul window, aligned to the back-half splits

# uniform chunk sizes (tapered head/tail chunks measured slower: the
# extra per-op overhead outweighs the fill/drain savings)
CHUNKS = [64] * 8
assert sum(CHUNKS) == SH

_cache = {}


def _build():
    f16 = mybir.dt.float16
    f32 = mybir.dt.float32
    Af = mybir.ActivationFunctionType
    nc = bacc.Bacc(
        "TRN2", target_bir_lowering=False, debug=False, num_devices=NCORES
    )
    xin = nc.dram_tensor("xin", [IN_FLAT], f16, kind="ExternalInput")
    ident = nc.dram_tensor("ident", [P * P], f16, kind="ExternalInput")
    yout = nc.dram_tensor("yout", [OUT_FLAT], f16, kind="ExternalOutput")

    K = len(CHUNKS)
    OFF = [sum(CHUNKS[:i]) for i in range(K)]   # frame offset per chunk

    with tile.TileContext(nc) as tc, ExitStack() as ctx:
        pconst = ctx.enter_context(tc.tile_pool(name="pconst", bufs=1))
        pin = ctx.enter_context(tc.tile_pool(name="pin", bufs=24))
        pmid = ctx.enter_context(tc.tile_pool(name="pmid", bufs=3))
        psm = ctx.enter_context(tc.tile_pool(name="psm", bufs=4))
        pout = ctx.enter_context(tc.tile_pool(name="pout", bufs=4))
        ppsum = ctx.enter_context(
            tc.tile_pool(name="ppsum", bufs=2, space="PSUM")
        )

        PF = 5  # input prefetch depth (chunks)

        def issue_in(k):
            # per-plane DMAs into per-plane tiles on the idle SP engine's
            # HWDGE ring: each plane's subtract can start as soon as its
            # own plane lands, with no Q7 descriptor-gen latency
            F = CHUNKS[k]
            FI = (F + 1) * NP
            ts = []
            for c in range(C):
                t = pin.tile([P, FI], f16)
                src = bass.AP(
                    xin,
                    c * IN_PLANE + OFF[k] * NP,
                    [[C * IN_PLANE, P], [1, FI]],
                )
                nc.sync.dma_start(t[:], src)
                ts.append(t)
            return ts

        in_tiles = [issue_in(0)]
        id_t = pconst.tile([P, P], f16)
        nc.sync.dma_start(id_t[:], bass.AP(ident, 0, [[P, P], [1, P]]))
        in_tiles += [issue_in(k) for k in range(1, PF)]

        def front(k):
            """subs + squares + PE sum for chunk k -> psum tile"""
            F = CHUNKS[k]
            FD = F * NP
            ins = in_tiles[k]
            # diff per plane: shift by one frame (26 elems = 52B) -> 2x
            d_t = pmid.tile([P, C * FD], f16)
            d3 = d_t[:].rearrange("p (c x) -> p c x", c=C)
            for c in range(C):
                nc.vector.tensor_sub(
                    d3[:, c], ins[c][:, NP:], ins[c][:, 0:FD]
                )
                # square in place; planes 0+1 are contiguous so they run
                # as one ACT op; plane 2 of some chunks runs as a DVE
                # multiply to balance ACT vs DVE busy time
                if c == 1:
                    nc.scalar.activation(
                        d_t[:, 0:2 * FD], d_t[:, 0:2 * FD], Af.Square
                    )
                elif c == 2:
                    if k in (0, 3):
                        nc.vector.tensor_mul(d3[:, c], d3[:, c], d3[:, c])
                    else:
                        nc.scalar.activation(d3[:, c], d3[:, c], Af.Square)

            # dist2 = sum over the 3 coord planes: identity matmuls
            # accumulating into PSUM on the otherwise-idle tensor engine
            ps_t = ppsum.tile([P, FD], f32)
            for w0 in range(0, FD, PSUM_W):
                w1 = min(w0 + PSUM_W, FD)
                for c in range(C):
                    nc.tensor.matmul(
                        ps_t[:, w0:w1],
                        id_t[:],
                        d3[:, c, w0:w1],
                        start=(c == 0),
                        stop=(c == C - 1),
                    )
            return ps_t

        def back(k, ps_t, splits=1):
            """sqrt + broadcast add + out DMA for chunk k; the final
            chunk runs in two halves so its output DMA overlaps the
            remaining adds instead of draining serially at the end"""
            F = CHUNKS[k]
            FD = F * NP
            ins = in_tiles[k]
            s_t = psm.tile([P, FD], f16)
            o_t = pout.tile([P, C * FD], f16)
            o3 = o_t[:].rearrange("p (c x) -> p c x", c=C)
            HW = FD // splits
            for h in range(splits):
                lo, hi = h * HW, (h + 1) * HW
                # dist = sqrt(dist2), draining PSUM (fp32) -> SBUF fp16
                nc.scalar.activation(s_t[:, lo:hi], ps_t[:, lo:hi], Af.Sqrt)
                # out_c = x_c + dist; DMA each plane as soon as ready
                for c in range(C):
                    nc.vector.tensor_add(
                        o3[:, c, lo:hi], ins[c][:, lo:hi], s_t[:, lo:hi]
                    )
                    dst = bass.AP(
                        yout,
                        c * OUT_PLANE + OFF[k] * NP + lo,
                        [[C * OUT_PLANE, P], [1, HW]],
                    )
                    nc.sync.dma_start(dst, o3[:, c, lo:hi])

        # one-chunk software pipeline: the back half of chunk k-1 issues
        # after the front half of chunk k, so sqrt(k-1) never makes the
        # in-order ACT queue stall on matmuls still in flight
        prev = None
        for k in range(K):
            ps_t = front(k)
            if k + PF < K:
                in_tiles.append(issue_in(k + PF))
            if prev is not None:
                back(k - 1, prev, splits=2)
            prev = ps_t
        back(K - 1, prev, splits=2)

    nc.compile()
    return nc


def kernel(x: np.ndarray, **_unused) -> np.ndarray:
    x = np.asarray(x)
    assert x.shape == (B, S, N, C), x.shape

    if "nc" not in _cache:
        _cache["nc"] = _build()
    nc = _cache["nc"]

    # Host-side repack: fp16, per (batch, half) partition a c-planar
    # [3, SH+1, 26] block; frame SH is the next real frame (half 0) or a
    # copy of the last frame (half 1) so the device-side distance at the
    # true sequence end is exactly 0.
    xh = np.ascontiguousarray(x).astype(np.float16)          # [B,S,25,3]
    ext = np.concatenate([xh, xh[:, -1:]], axis=1)           # [B,S+1,25,3]
    h0 = ext[:, 0:SH + 1]                                    # [B,513,25,3]
    h1 = ext[:, SH:S + 1]                                    # [B,513,25,3]
    hv = np.stack([h0, h1], axis=1)                          # [B,2,513,25,3]
    pl = np.transpose(hv, (0, 1, 4, 2, 3))                   # [B,2,3,513,25]
    buf = np.zeros((B, H, C, SH + 1, NP), np.float16)
    buf[..., :N] = pl

    eye = np.eye(P, dtype=np.float16).reshape(P * P)
    in_maps = [
        {
            "xin": buf[ci * BC:(ci + 1) * BC].reshape(IN_FLAT),
            "ident": eye,
        }
        for ci in range(NCORES)
    ]

    res = run_bass_kernel_spmd(nc, in_maps, core_ids=list(range(NCORES)))
    _cache["last_results"] = res

    out = np.empty((B, S, N, C), dtype=np.float32)
    for ci in range(NCORES):
        y = np.asarray(res.results[ci]["yout"]).reshape(BC, H, C, SH, NP)
        y = y[..., :N]                                       # strip node pad
        y = np.transpose(y, (0, 1, 3, 4, 2))                 # [BC,2,SH,25,3]
        out[ci * BC:(ci + 1) * BC] = y.reshape(BC, S, N, C).astype(np.float32)
    return out


# revision 43
# speedup vs baseline: 1.0516x; 1.0516x over previous
"""Trainium2 Bass kernel for nn_PositionalEncoding_61151744360729.

out[b, s, n, :] = x[b, s, n, :] + ||x[b, s+1, n, :] - x[b, s, n, :]||_2
(with distance 0 at s = S-1).

Sharding: data-parallel on batch across 8 NeuronCores (64 batches/core).

Device layout: fp16 end-to-end, c-planar. Host repacks x to, per
(batch, seq-half) partition, [3 coord planes][SH+1 frames][26 nodes]
(nodes padded 25->26 so the one-frame shift is 52B = 4B-aligned and all
DVE tensor_tensor ops hit the 2x perf mode; fp16 I/O halves HBM traffic
vs fp32).

Engine split per chunk: DVE does the 3 per-plane frame-shift subtracts
and the 3 per-plane broadcast-adds; ACT squares each plane (packing
26->25) and takes the final sqrt; the 3-plane sum runs on the
otherwise-idle PE as identity matmuls accumulating into PSUM (the sqrt
doubles as the PSUM->SBUF drain); outputs DMA out per plane. A
one-chunk software pipeline keeps the in-order ACT queue from stalling
on in-flight matmuls, and chunk sizes taper (32 frames at the ends,
64 mid) to shrink pipeline fill and the final DMA drain.
"""

import sys
from contextlib import ExitStack

for _p in ("/opt/trn_rl_repo", "/root/.axon_site/_ro/trn_rl_repo"):
    if _p not in sys.path:
        sys.path.insert(0, _p)

import numpy as np

import concourse.bass as bass
import concourse.tile as tile
from concourse import bacc, mybir
from concourse.bass_utils import run_bass_kernel_spmd

B, S, N, C = 512, 1024, 25, 3
NCORES = 8
BC = B // NCORES           # 64 batches per core
H = 2                      # sequence halves -> 128 partitions
SH = S // H                # 512 frames per half
P = H * BC                 # 128 partitions
NP = 26                    # nodes padded to 26 (4B-aligned frame stride)
IN_PLANE = (SH + 1) * NP   # input elems per coord plane per partition
OUT_PLANE = SH * NP        # output elems per plane per partition
IN_FLAT = P * C * IN_PLANE
OUT_FLAT = P * C * OUT_PLANE
PSUM_W = 512               # one PSUM bank of fp32 per matmul window
# (windows must be bank-aligned: accumulation across a bank edge corrupts)

# uniform chunk sizes (tapered head/tail chunks measured slower: the
# extra per-op overhead outweighs the fill/drain savings)
CHUNKS = [64] * 8
assert sum(CHUNKS) == SH

_cache = {}


def _build():
    f16 = mybir.dt.float16
    f32 = mybir.dt.float32
    Af = mybir.ActivationFunctionType
    nc = bacc.Bacc(
        "TRN2", target_bir_lowering=False, debug=False, num_devices=NCORES
    )
    xin = nc.dram_tensor("xin", [IN_FLAT], f16, kind="ExternalInput")
    ident = nc.dram_tensor("ident", [P * P], f16, kind="ExternalInput")
    yout = nc.dram_tensor("yout", [OUT_FLAT], f16, kind="ExternalOutput")

    K = len(CHUNKS)
    OFF = [sum(CHUNKS[:i]) for i in range(K)]   # frame offset per chunk

    with tile.TileContext(nc) as tc, ExitStack() as ctx:
        pconst = ctx.enter_context(tc.tile_pool(name="pconst", bufs=1))
        pin = ctx.enter_context(tc.tile_pool(name="pin", bufs=24))
        pmid = ctx.enter_context(tc.tile_pool(name="pmid", bufs=3))
        psm = ctx.enter_context(tc.tile_pool(name="psm", bufs=4))
        pout = ctx.enter_context(tc.tile_pool(name="pout", bufs=4))
        ppsum = ctx.enter_context(
            tc.tile_pool(name="ppsum", bufs=2, space="PSUM")
        )

        PF = 5  # input prefetch depth (chunks)

        def issue_in(k):
            # per-plane DMAs into per-plane tiles on the idle SP engine's
            # HWDGE ring: each plane's subtract can start as soon as its
            # own plane lands, with no Q7 descriptor-gen latency
            F = CHUNKS[k]
            FI = (F + 1) * NP
            ts = []
            for c in range(C):
                t = pin.tile([P, FI], f16)
                src = bass.AP(
                    xin,
                    c * IN_PLANE + OFF[k] * NP,
                    [[C * IN_PLANE, P], [1, FI]],
                )
                nc.sync.dma_start(t[:], src)
                ts.append(t)
            return ts

        in_tiles = [issue_in(0)]
        id_t = pconst.tile([P, P], f16)
        nc.sync.dma_start(id_t[:], bass.AP(ident, 0, [[P, P], [1, P]]))
        in_tiles += [issue_in(k) for k in range(1, PF)]

        def front(k):
            """subs + squares + PE sum for chunk k -> psum tile"""
            F = CHUNKS[k]
            FD = F * NP
            ins = in_tiles[k]
            # diff per plane: shift by one frame (26 elems = 52B) -> 2x
            d_t = pmid.tile([P, C * FD], f16)
            d3 = d_t[:].rearrange("p (c x) -> p c x", c=C)
            for c in range(C):
                nc.vector.tensor_sub(
                    d3[:, c], ins[c][:, NP:], ins[c][:, 0:FD]
                )
                # square in place; planes 0+1 are contiguous so they run
                # as one ACT op; plane 2 of some chunks runs as a DVE
                # multiply to balance ACT vs DVE busy time
                if c == 1:
                    nc.scalar.activation(
                        d_t[:, 0:2 * FD], d_t[:, 0:2 * FD], Af.Square
                    )
                elif c == 2:
                    if k in (0, 3):
                        nc.vector.tensor_mul(d3[:, c], d3[:, c], d3[:, c])
                    else:
                        nc.scalar.activation(d3[:, c], d3[:, c], Af.Square)

            # dist2 = sum over the 3 coord planes: identity matmuls
            # accumulating into PSUM on the otherwise-idle tensor engine
            ps_t = ppsum.tile([P, FD], f32)
            for w0 in range(0, FD, PSUM_W):
                w1 = min(w0 + PSUM_W, FD)
                for c in range(C):
                    nc.tensor.matmul(
                        ps_t[:, w0:w1],
                        id_t[:],
                        d3[:, c, w0:w1],
                        start=(c == 0),
                        stop=(c == C - 1),
                    )
            return ps_t

        def back(k, ps_t, splits=1):
            """sqrt + broadcast add + out DMA for chunk k; the final
            chunk runs in two halves so its output DMA overlaps the
            remaining adds instead of draining serially at the end"""
            F = CHUNKS[k]
            FD = F * NP
            ins = in_tiles[k]
            s_t = psm.tile([P, FD], f16)
            o_t = pout.tile([P, C * FD], f16)
            o3 = o_t[:].rearrange("p (c x) -> p c x", c=C)
            HW = FD // splits
            for h in range(splits):
                lo, hi = h * HW, (h + 1) * HW
                # dist = sqrt(dist2), draining PSUM (fp32) -> SBUF fp16
                nc.scalar.activation(s_t[:, lo:hi], ps_t[:, lo:hi], Af.Sqrt)
                # out_c = x_c + dist; DMA each plane as soon as ready
                for c in range(C):
                    nc.vector.tensor_add(
                        o3[:, c, lo:hi], ins[c][:, lo:hi], s_t[:, lo:hi]
                    )
                    dst = bass.AP(
                        yout,
                        c * OUT_PLANE + OFF[k] * NP + lo,
                        [[C * OUT_PLANE, P], [1, HW]],
                    )
                    nc.sync.dma_start(dst, o3[:, c, lo:hi])

        # one-chunk software pipeline: the back half of chunk k-1 issues
        # after the front half of chunk k, so sqrt(k-1) never makes the
        # in-order ACT queue stall on matmuls still in flight
        prev = None
        for k in range(K):
            ps_t = front(k)
            if k + PF < K:
                in_tiles.append(issue_in(k + PF))
            if prev is not None:
                back(k - 1, prev, splits=2)
            prev = ps_t
        back(K - 1, prev, splits=2)

    nc.compile()
    return nc


def kernel(x: np.ndarray, **_unused) -> np.ndarray:
    x = np.asarray(x)
    assert x.shape == (B, S, N, C), x.shape

    if "nc" not in _cache:
        _cache["nc"] = _build()
    nc = _cache["nc"]

    # Host-side repack: fp16, per (batch, half) partition a c-planar
    # [3, SH+1, 26] block; frame SH is the next real frame (half 0) or a
    # copy of the last frame (half 1) so the device-side distance at the
    # true sequence end is exactly 0.
    xh = np.ascontiguousarray(x).astype(np.float16)          # [B,S,25,3]
    ext = np.concatenate([xh, xh[:, -1:]], axis=1)           # [B,S+1,25,3]
    h0 = ext[:, 0:SH + 1]                                    # [B,513,25,3]
    h1 = ext[:, SH:S + 1]                                    # [B,513,25,3]
    hv = np.stack([h0, h1], axis=1)                          # [B,2,513,25,3]
    pl = np.transpose(hv, (0, 1, 4, 2, 3))                   # [B,2,3,513,25]
    buf = np.zeros((B, H, C, SH + 1, NP), np.float16)
    buf[..., :N] = pl

    eye = np.eye(P, dtype=np.float16).reshape(P * P)
    in_maps = [
        {
            "xin": buf[ci * BC:(ci + 1) * BC].reshape(IN_FLAT),
            "ident": eye,
        }
        for ci in range(NCORES)
    ]

    res = run_bass_kernel_spmd(nc, in_maps, core_ids=list(range(NCORES)))
    _cache["last_results"] = res

    out = np.empty((B, S, N, C), dtype=np.float32)
    for ci in range(NCORES):
        y = np.asarray(res.results[ci]["yout"]).reshape(BC, H, C, SH, NP)
        y = y[..., :N]                                       # strip node pad
        y = np.transpose(y, (0, 1, 3, 4, 2))                 # [BC,2,SH,25,3]
        out[ci * BC:(ci + 1) * BC] = y.reshape(BC, S, N, C).astype(np.float32)
    return out
